# revision 1
# baseline (speedup 1.0000x reference)
"""Trainium2 Bass kernel for per-position FC decoder stack.

out[b, o3, p] = W3[p] @ (W2[p] @ (W1[p] @ glf[b] + b1[p]) + b2[p]) + b3[p]

Shapes: glf [32, 512, 1], W1 [2048, 32, 512], W2 [2048, 8, 32], W3 [2048, 3, 8].
All layers are linear, so we fold W2/b2/b1 into a per-position affine map
M1aug[p] = [W2@W1 | W2@b1 + b2]  ([8, 513]).  The key trick: compute M1^T
DIRECTLY by using W1 (natural layout, 128-row chunks as stationary operand)
against a zero-padded block-diagonal W2^T (moving operand, N=256 so float32r
runs at full PE rate).  The 128 MiB W1 is never transposed and M1 never needs
a PE transpose pass -- PSUM drains straight into the M1^T layout stage 2 wants.

Stage 2 applies M1aug^T to glf^T (accumulating float32r matmuls); stage 3
(the tiny [3,8] per-position maps) runs on GpSimd/DVE with strided APs.

Sharding: positions (2048) split across 8 cores; glf replicated.
"""

import sys

if "/opt/trn_rl_repo" not in sys.path:
    sys.path.insert(0, "/opt/trn_rl_repo")

import numpy as np

# Problem constants (hardcoded per contest contract)
P_FULL = 2048
NCORES = 8
PP = P_FULL // NCORES  # 256 positions per core
B = 32
I = 512
O1 = 32
O2 = 8
O3 = 3
NT = 16    # t-blocks of 16 positions (one [128, 2048] W1 DMA each)
NTT = 8    # tt-blocks of 32 positions (one psum group each)
NG = 64    # groups of 4 positions

_CACHE = {}


def _build_nc():
    import concourse.bass as bass
    import concourse.mybir as mybir
    import concourse.tile as tile
    from concourse import bacc
    from concourse.masks import make_identity

    F32 = mybir.dt.float32
    F32R = mybir.dt.float32r
    ADD = mybir.AluOpType.add
    MULT = mybir.AluOpType.mult
    AX_X = mybir.AxisListType.X
    IDENT = mybir.ActivationFunctionType.Identity

    nc = bacc.Bacc(
        "TRN2", target_bir_lowering=False, debug=False, num_devices=NCORES
    )
    W1 = nc.declare_dram_parameter("W1", [PP, O1, I], F32R, isOutput=False)
    b1 = nc.declare_dram_parameter("b1", [PP, O1], F32R, isOutput=False)
    W2 = nc.declare_dram_parameter("W2", [PP, O2, O1], F32, isOutput=False)
    b2 = nc.declare_dram_parameter("b2", [PP, O2], F32R, isOutput=False)
    W3 = nc.declare_dram_parameter("W3", [PP, O3, O2], F32, isOutput=False)
    b3 = nc.declare_dram_parameter("b3", [PP, O3], F32, isOutput=False)
    glf = nc.declare_dram_parameter("glf", [B, I], F32, isOutput=False)
    out = nc.declare_dram_parameter("out", [B, O3, PP], F32, isOutput=True)

    with tile.TileContext(nc) as tc:
        with (
            tc.tile_pool(name="persist", bufs=1) as pp,
            tc.tile_pool(name="w1s", bufs=8) as w1p,
            tc.tile_pool(name="l3", bufs=2) as l3p,
            tc.tile_pool(name="pst", bufs=4, space="PSUM") as pstp,
            tc.tile_pool(name="psb", bufs=2, space="PSUM") as psbp,
            tc.tile_pool(name="psy", bufs=2, space="PSUM") as psyp,
        ):
            # ---------------- constants / small-input prep ----------------
            # Rotating block-diag W2T buffers (zero pattern identical per cq):
            # zero once, overwrite only the diagonal blocks each round.
            bd4 = [
                pp.tile([128, 4096 + 280], F32R, tag=f"bd4{i}", name=f"bd4{i}") for i in range(2)
            ]
            zsrc = pp.tile([128, 256], F32, tag="zsrc")
            nc.vector.memset(zsrc, 0.0)
            for i in range(2):
                nc.vector.tensor_copy(
                    bd4[i][:, 0 : 4096 + 256].rearrange("q (g c) -> q g c", c=256),
                    zsrc[:, :].rearrange("q (g c) -> q g c", g=1).broadcast_to(
                        [128, 17, 256]
                    ),
                )

            ident = pp.tile([128, 128], F32, tag="ident")
            make_identity(nc, ident)

            # glf [32, 512] -> glfT chunks: glfT[:, 32k:32k+32] = glf[:, 128k:+128].T
            glf_sb = pp.tile([B, I], F32, tag="glf")
            nc.scalar.dma_start(out=glf_sb, in_=glf[:])
            glfT = pp.tile([128, 128], F32R, tag="glfT")
            for k in range(4):
                pt = pstp.tile([128, 256], F32, tag="pst")
                nc.tensor.transpose(
                    pt[0:128, 0:B], glf_sb[:, 128 * k : 128 * (k + 1)], ident[0:B, 0:B]
                )
                nc.vector.tensor_copy(glfT[:, 32 * k : 32 * k + 32], pt[0:128, 0:B])

            ones_sb = pp.tile([1, B], F32R, tag="ones")
            ones_f32 = pp.tile([1, B], F32, tag="ones32")
            nc.vector.memset(ones_f32, 1.0)
            nc.vector.tensor_copy(ones_sb, ones_f32)

            # W2 natural [(p,o2), o1] = [2048, 32] -> 16 chunks [128, 32]
            w2nat = pp.tile([128, NT * O1], F32, tag="w2nat")  # [128, 512]
            nc.sync.dma_start(
                out=w2nat[:].rearrange("q (c o) -> q c o", c=NT),
                in_=W2[:].rearrange("p o2 o1 -> (p o2) o1").rearrange(
                    "(c q) o -> q c o", q=128
                ),
            )

            # b1_sb[q, g] = b1_flat[128 g + q] = b1[4g + (q//32), q%32]
            # natural load + PE transpose + strided psum->sbuf copies
            b1nat = pp.tile([128, 2 * O1], F32, tag="b1nat")
            nc.scalar.dma_start(
                out=b1nat[:].rearrange("q (h o) -> q h o", h=2),
                in_=b1[:].bitcast(F32).rearrange("(h q) o -> q h o", q=128),
            )
            b1_sb = pp.tile([128, NG + 1], F32R, tag="b1")
            nc.vector.memset(b1_sb[:, NG : NG + 1].bitcast(F32), 0.0)
            ptb = pstp.tile([128, 256], F32, tag="pst")
            nc.tensor.transpose(
                ptb[0:64, 0:128], b1nat[:, 0:64], ident[0:128, 0:128]
            )
            for h in range(2):
                for j in range(4):
                    nc.vector.tensor_copy(
                        b1_sb[32 * j : 32 * (j + 1), 32 * h : 32 * (h + 1)],
                        ptb[32 * h : 32 * (h + 1), 0:128].rearrange(
                            "q (g f) -> q g f", f=4
                        )[:, :, j],
                    )
            # b2row: contiguous flat (p,o2) row; added to Y2 via a k=1 matmul
            b2row = pp.tile([1, PP * O2], F32R, tag="b2row")
            nc.scalar.dma_start(
                out=b2row,
                in_=b2[:].rearrange("p o -> (p o)").rearrange("(o f) -> o f", o=1),
            )

            # dummy PE reads so later matmuls don't accumulate waits
            ptd = pstp.tile([128, 256], F32, tag="pst")
            nc.tensor.transpose(ptd[0:1, 0:128], bd4[0][:, 0:1].bitcast(F32), ident)
            ptd2 = pstp.tile([128, 256], F32, tag="pst")
            nc.tensor.transpose(ptd2[0:1, 0:128], b1_sb[:, 0:1].bitcast(F32), ident)

            # W3 / b3 broadcast across the 32 batch partitions (GpSimd), in place
            w3bc = pp.tile([B, PP * O3 * O2], F32, tag="w3bc")  # [32, 6144]
            nc.scalar.dma_start(
                out=w3bc[0:1, :],
                in_=W3[:].rearrange("p x o -> (p x o)").rearrange(
                    "(o f) -> o f", o=1
                ),
            )
            nc.gpsimd.partition_broadcast(w3bc, w3bc[0:1, :], channels=B)
            b3bc = pp.tile([B, PP * O3], F32, tag="b3bc")  # [32, 768]
            nc.scalar.dma_start(
                out=b3bc[0:1, :],
                in_=b3[:].rearrange("p x -> (p x)").rearrange("(o f) -> o f", o=1),
            )
            nc.gpsimd.partition_broadcast(b3bc, b3bc[0:1, :], channels=B)

            # Persistent M1^T / beff^T:
            # m1T region k (i-chunk) at cols [2048k, 2048(k+1)); col = flat (p,o2)
            m1T = pp.tile([128, 4 * PP * O2], F32R, tag="m1T")  # [128, 8192]
            beffT = pp.tile([1, PP * O2], F32R, tag="beffT")  # [1, 2048]

            out_sb = pp.tile([B, O3 * PP], F32, tag="outsb")  # [32, 768], (o3, p)

            def tail_chunk(cc):
                """Stage 2+3 for (p,o2) cols [256 cc, +256) = positions [32 cc, +32)."""
                py = psyp.tile([B, 256], F32, tag="py")
                nc.tensor.matmul(
                    py,
                    lhsT=ones_sb,
                    rhs=beffT[0:1, 256 * cc : 256 * (cc + 1)],
                    start=True,
                    stop=False,
                )
                nc.tensor.matmul(
                    py,
                    lhsT=ones_sb,
                    rhs=b2row[0:1, 256 * cc : 256 * (cc + 1)],
                    start=False,
                    stop=False,
                )
                for k in range(4):
                    nc.tensor.matmul(
                        py,
                        lhsT=glfT[:, 32 * k : 32 * (k + 1)],
                        rhs=m1T[:, 2048 * k + 256 * cc : 2048 * k + 256 * (cc + 1)],
                        start=False,
                        stop=(k == 3),
                    )
                # stage 3 straight from PSUM: one mult (o3-broadcast) + one reduce
                POS = 32
                p0 = POS * cc
                py3 = py[:, :].rearrange(
                    "q (x p c) -> q x p c", x=1, p=POS
                ).broadcast_to([B, O3, POS, O2])
                w3v = w3bc[:, :].rearrange("q (p x c) -> q x p c", p=PP, x=O3)[
                    :, :, p0 : p0 + POS, :
                ]
                prod = l3p.tile([B, O3 * POS * O2], F32, tag="prod")
                prodv = prod[:, :].rearrange("q (x p c) -> q x p c", x=O3, p=POS)
                nc.vector.tensor_tensor(prodv, py3, w3v, MULT)
                outv = out_sb[:, :].rearrange("q (x p) -> q x p", x=O3)[
                    :, :, p0 : p0 + POS
                ]
                nc.vector.tensor_reduce(outv, prodv, AX_X, ADD)
                b3v = b3bc[:, :].rearrange("q (p x) -> q x p", x=O3)[
                    :, :, p0 : p0 + POS
                ]
                nc.vector.tensor_tensor(outv, outv, b3v, ADD)
                nc.scalar.dma_start(
                    out=out[:].rearrange("b x p -> b x p")[:, :, p0 : p0 + POS],
                    in_=outv,
                )

            # ---------------- stage 1: M1^T directly via operand swap ----------------
            # per tt (32 positions): 4 i-chunks x 8 accumulating matmuls
            #   out[i, (p_loc, o2)] += sum_k W1chunk[k, i] * bd4band[k, n]
            # Block-diag W2T band (tt, u) lives in bd4[tt % 2] at local cols
            # [2048 (tt%2) + 256 u, +256); nonzeros at 288 u + 8 j + o2.
            w1tiles = {}
            for cq in range(4):
                # transpose W2 chunk cq and scatter diagonal blocks into bd4[cq%2]
                buf = bd4[cq % 2]
                pt = pstp.tile([128, 256], F32, tag="pst")
                nc.tensor.transpose(
                    pt[0:128, 0:128],
                    w2nat[:, 128 * cq : 128 * (cq + 1)],
                    ident[0:128, 0:128],
                )
                for cl in range(4):
                    for j in range(4):
                        base = 2048 * (cl // 2) + 1152 * (cl % 2) + 8 * j
                        dst = buf[
                            32 * j : 32 * (j + 1), base : base + 1152
                        ].rearrange("q (v r) -> q v r", r=288)[:, :, 0:8]
                        srcv = pt[
                            32 * cl : 32 * (cl + 1), 0:128
                        ].rearrange("q (v r) -> q v r", r=32)[
                            :, :, 8 * j : 8 * j + 8
                        ]
                        nc.vector.tensor_copy(dst, srcv)

                for tth in range(2):
                    tt = 2 * cq + tth
                    loc = 2048 * (tt % 2)
                    for half_t in range(2):
                        t = 2 * tt + half_t
                        w1t = w1p.tile([128, 4 * I], F32R, tag="w1t")
                        w1tiles[t] = w1t
                        w1src = (
                            W1[:]
                            .rearrange("p o i -> (p o) i")[512 * t : 512 * (t + 1), :]
                            .rearrange("(u q) i -> q u i", q=128)
                        )
                        w1dst = w1t[:].rearrange("q (u i) -> q u i", u=4)
                        if t >= 14:
                            # split the last tiles so their matmul chains can
                            # start before the full tile lands
                            for ic in range(4):
                                nc.sync.dma_start(
                                    out=w1dst[:, :, 128 * ic : 128 * (ic + 1)],
                                    in_=w1src[:, :, 128 * ic : 128 * (ic + 1)],
                                )
                        else:
                            nc.sync.dma_start(out=w1dst, in_=w1src)
                    for c in range(4):
                        pst = pstp.tile([128, 256], F32, tag="pst")
                        for u in range(NTT):
                            w1t = w1tiles[2 * tt + u // 4]
                            lhsT = w1t[:].rearrange("q (v i) -> q v i", v=4)[
                                :, u % 4, 128 * c : 128 * (c + 1)
                            ]
                            nc.tensor.matmul(
                                pst,
                                lhsT=lhsT,
                                rhs=buf[:, loc + 256 * u : loc + 256 * (u + 1)],
                                start=(u == 0),
                                stop=(u == NTT - 1),
                            )
                        dst_m1 = m1T[
                            :, 2048 * c + 256 * tt : 2048 * c + 256 * (tt + 1)
                        ]
                        if c % 2 == 0:
                            nc.scalar.copy(dst_m1, pst)
                        else:
                            nc.vector.tensor_copy(dst_m1, pst)

                    # aug (bias) rows: beffT[32g:+32] = b1_g^T @ W2T-block_g
                    for half_t in range(2):
                        t = 2 * tt + half_t
                        for v in range(4):
                            g = 4 * t + v
                            u = g % 8  # band index within tt
                            psa = psbp.tile([2, 32], F32, tag="psb")
                            nc.tensor.matmul(
                                psa,
                                lhsT=b1_sb[:, g : g + 2],
                                rhs=buf[:, loc + 288 * u : loc + 288 * u + 32],
                                start=True,
                                stop=True,
                            )
                            nc.scalar.copy(
                                beffT[0:1, 32 * g : 32 * (g + 1)], psa[0:1, :]
                            )

                    tail_chunk(tt)


    nc.compile()
    return nc


def _get_nc():
    if "nc" not in _CACHE:
        _CACHE["nc"] = _build_nc()
    return _CACHE["nc"]


def _make_in_maps(inputs):
    glf = np.ascontiguousarray(
        np.asarray(inputs["glf"], dtype=np.float32).reshape(B, I)
    )
    ins = {k: np.asarray(inputs[k], dtype=np.float32) for k in
           ("W1", "b1", "W2", "b2", "W3", "b3")}
    in_maps = []
    for c in range(NCORES):
        sl = slice(c * PP, (c + 1) * PP)
        in_maps.append(
            {
                "W1": np.ascontiguousarray(ins["W1"][sl]),
                "b1": np.ascontiguousarray(ins["b1"][sl]),
                "W2": np.ascontiguousarray(ins["W2"][sl]),
                "b2": np.ascontiguousarray(ins["b2"][sl]),
                "W3": np.ascontiguousarray(ins["W3"][sl]),
                "b3": np.ascontiguousarray(ins["b3"][sl]),
                "glf": glf,
            }
        )
    return in_maps


def run(inputs, trace=False):
    """Run on the 8 NeuronCores; returns (out_full, BassKernelResults)."""
    from concourse.bass_utils import run_bass_kernel_spmd

    nc = _get_nc()
    res = run_bass_kernel_spmd(
        nc, _make_in_maps(inputs), list(range(NCORES)), trace=trace
    )
    out_full = np.empty((B, O3, P_FULL), dtype=np.float32)
    for c in range(NCORES):
        out_full[:, :, c * PP : (c + 1) * PP] = res.results[c]["out"]
    return out_full, res


def kernel(**inputs):
    out, _ = run(inputs, trace=False)
    return out



# revision 8
# speedup vs baseline: 1.9517x; 1.9517x over previous
"""Trainium2 Bass kernel for per-position FC decoder stack.

out[b, o3, p] = W3[p] @ (W2[p] @ (W1[p] @ glf[b] + b1[p]) + b2[p]) + b3[p]

Shapes: glf [32, 512, 1], W1 [2048, 32, 512], W2 [2048, 8, 32], W3 [2048, 3, 8].

Pipeline (per core, 256 positions, fp16 weights/activations, f32 psum):
  stage 1  M1T[i, (p,o2)] = W2@W1 fold via block-diag W2T moving operand.
           One matmul per (i-chunk c, 4-position group g): lhsT = natural
           W1 rows [128=(4p,32o1), 128=i], rhs = block-diag W2T [128, 32].
  beff     beff[(p,o2)] = W2@b1 + b2 via the same block-diag rhs.
  stage 2  Y2T[(p,o2), b] = M1T^T glf^T + beff: lhsT = M1T chunk, rhs = glfT.
  stage 3  OUT[b, (p,o3)] = block-diag W3T applied to Y2T: lhsT = Y2T chunk,
           rhs = block-diag W3T [128=(16p,8o2), 48=(16p,3o3)].
  The 16 W1 DMA chunks (1 per 16 positions) pace a 16-unit software pipeline;
  everything else hides under the W1 HBM stream.

Sharding: positions (2048) split across 8 cores; glf replicated.
"""

import sys

if "/opt/trn_rl_repo" not in sys.path:
    sys.path.insert(0, "/opt/trn_rl_repo")

import numpy as np

# Problem constants (hardcoded per contest contract)
P_FULL = 2048
NCORES = 8
PP = P_FULL // NCORES  # 256 positions per core
B = 32
I = 512
O1 = 32
O2 = 8
O3 = 3
NT = 16  # pipeline units of 16 positions (one W1 DMA chunk each)

_CACHE = {}


def _build_nc():
    import concourse.bass as bass
    import concourse.mybir as mybir
    import concourse.tile as tile
    from concourse import bacc

    F32 = mybir.dt.float32
    F16 = mybir.dt.float16
    ADD = mybir.AluOpType.add

    nc = bacc.Bacc(
        "TRN2", target_bir_lowering=False, debug=False, num_devices=NCORES
    )
    # host-prepped (layout/dtype only) inputs
    W1h = nc.declare_dram_parameter("W1h", [NT, 128, 4 * I], F16, isOutput=False)
    W2T = nc.declare_dram_parameter("W2T", [O1, PP * O2], F16, isOutput=False)
    W3BD = nc.declare_dram_parameter("W3BD", [128, NT * 48], F16, isOutput=False)
    GLFT = nc.declare_dram_parameter("GLFT", [128, 4 * B], F16, isOutput=False)
    B1C = nc.declare_dram_parameter("B1C", [128, 64], F16, isOutput=False)
    B2R = nc.declare_dram_parameter("B2R", [1, PP * O2], F16, isOutput=False)
    B3R = nc.declare_dram_parameter("B3R", [1, O3 * PP], F32, isOutput=False)
    OUT = nc.declare_dram_parameter("OUT", [B, O3, PP], F32, isOutput=True)

    with tile.TileContext(nc) as tc:
        with (
            tc.tile_pool(name="persist", bufs=1) as pp,
            tc.tile_pool(name="pst", bufs=2, space="PSUM") as pstp,
            tc.tile_pool(name="y2ps", bufs=2, space="PSUM") as y2pp,
            tc.tile_pool(name="befps", bufs=2, space="PSUM") as befpp,
            tc.tile_pool(name="outps", bufs=2, space="PSUM") as outpp,
        ):
            # ---------------- small inputs (SP queue) ----------------
            glfT = pp.tile([128, 4 * B], F16, tag="glfT")
            nc.sync.dma_start(out=glfT, in_=GLFT[:])
            w2t_sb = pp.tile([O1, PP * O2], F16, tag="w2t")
            nc.sync.dma_start(out=w2t_sb, in_=W2T[:])
            w3bd = pp.tile([128, NT * 48], F16, tag="w3bd")
            nc.sync.dma_start(out=w3bd, in_=W3BD[:])
            b1c = pp.tile([128, 64], F16, tag="b1c")
            nc.sync.dma_start(out=b1c, in_=B1C[:])
            b2r = pp.tile([1, PP * O2], F16, tag="b2r")
            nc.sync.dma_start(out=b2r, in_=B2R[:])
            b3bc = pp.tile([B, O3 * PP], F32, tag="b3bc")
            nc.sync.dma_start(out=b3bc[0:1, :], in_=B3R[:])
            nc.gpsimd.partition_broadcast(b3bc, b3bc[0:1, :], channels=B)

            ones_sb = pp.tile([1, B], F16, tag="ones")
            nc.gpsimd.memset(ones_sb, 1.0)

            # block-diag W2T [128=(4p,32o1), (g, 32=(4p,8o2))]
            w2bd = pp.tile([128, 64 * 32], F16, tag="w2bd")
            nc.vector.memset(w2bd, 0.0)
            for j in range(4):
                nc.vector.tensor_copy(
                    w2bd[:, :].rearrange("q (g n) -> q g n", n=32)[
                        32 * j : 32 * (j + 1), :, 8 * j : 8 * j + 8
                    ],
                    w2t_sb[:, :].rearrange("q (g r) -> q g r", r=32)[
                        :, :, 8 * j : 8 * j + 8
                    ],
                )
            # ---------------- beff = W2 @ b1 + b2 (PE + DVE) ----------------
            beff_sb = pp.tile([1, PP * O2], F16, tag="beff")
            for q in range(4):
                bps = befpp.tile([1, 512], F32, tag="befps")
                for gg in range(16):
                    g = 16 * q + gg
                    nc.tensor.matmul(
                        bps[0:1, 32 * gg : 32 * (gg + 1)],
                        lhsT=b1c[:, g : g + 1],
                        rhs=w2bd[:, 32 * g : 32 * (g + 1)],
                        start=(gg == 0),
                        stop=(gg == 15),
                    )
                nc.vector.tensor_tensor(
                    beff_sb[0:1, 512 * q : 512 * (q + 1)],
                    bps[0:1, :],
                    b2r[0:1, 512 * q : 512 * (q + 1)],
                    ADD,
                )

            # ---------------- W1 chunks + persistent unit tiles ----------------
            w1t = [pp.tile([128, 4 * I], F16, tag=f"w1_{t}", name=f"w1_{t}")
                   for t in range(NT)]
            m1t = [pp.tile([128, 4 * 128], F16, tag=f"m1_{t}", name=f"m1_{t}")
                   for t in range(NT)]
            y2sb = [pp.tile([128, B], F16, tag=f"y2_{t}", name=f"y2_{t}")
                    for t in range(NT)]
            out_sb = pp.tile([B, O3 * PP], F32, tag="outsb")
            outps_tiles = {}

            def dma_w1(t):
                eng = nc.sync if t % 2 == 0 else nc.scalar
                eng.dma_start(out=w1t[t], in_=W1h[t])

            def stage1(t):
                """16 matmuls -> pst[t] [128, (c 4, 32)] = M1T cols of unit t."""
                pst = pstp.tile([128, 512], F32, tag="pst")
                for u in range(4):
                    g = 4 * t + u
                    for c in range(4):
                        nc.tensor.matmul(
                            pst[:, 128 * c + 32 * u : 128 * c + 32 * (u + 1)],
                            lhsT=w1t[t][:, :].rearrange(
                                "q (u i) -> q u i", u=4
                            )[:, u, 128 * c : 128 * (c + 1)],
                            rhs=w2bd[:, 32 * g : 32 * (g + 1)],
                            start=(u == 0 and c == 0),
                            stop=(u == 3 and c == 3),
                        )
                return pst

            def drain(t, pst):
                """psum [128, (c, 128)] -> sbuf m1t[t] fp16."""
                eng = nc.scalar.copy if t % 2 == 0 else nc.vector.tensor_copy
                eng(m1t[t][:, :], pst[:, :])

            def stage2(t):
                """Y2T[t] [128=(16p,8o2), 32=b] = M1T^T glfT + beff."""
                y2 = y2pp.tile([128, B], F32, tag="y2ps")
                for c in range(4):
                    nc.tensor.matmul(
                        y2,
                        lhsT=m1t[t][:, :].rearrange("q (c m) -> q c m", c=4)[
                            :, c, :
                        ],
                        rhs=glfT[:, :].rearrange("q (c b) -> q c b", c=4)[:, c, :],
                        start=(c == 0),
                        stop=False,
                    )
                nc.tensor.matmul(
                    y2,
                    lhsT=beff_sb[0:1, 128 * t : 128 * (t + 1)],
                    rhs=ones_sb[0:1, :],
                    start=False,
                    stop=True,
                )
                eng = nc.vector.tensor_copy if t % 2 == 0 else nc.scalar.copy
                eng(y2sb[t][:, :], y2)

            def stage3(t):
                """OUT chunk [32, 48=(16p,3o3)] into outps bank k = t//4."""
                k = t // 4
                if t % 4 == 0:
                    outps_tiles[k] = outpp.tile(
                        [B, 4 * 48], F32, tag="outps", name=f"outps_{k}"
                    )
                nc.tensor.matmul(
                    outps_tiles[k][:, 48 * (t % 4) : 48 * (t % 4 + 1)],
                    lhsT=y2sb[t][:, :],
                    rhs=w3bd[:, 48 * t : 48 * (t + 1)],
                    start=(t % 4 == 0),
                    stop=(t % 4 == 3),
                )

            def finish(k):
                """Add b3, fix layout [32,(p64,o3)]->[32,(o3,p64)], for t-bank k."""
                ops = outps_tiles.pop(k)
                nc.vector.tensor_tensor(
                    out_sb[:, :].rearrange("q (x p) -> q x p", x=O3)[
                        :, :, 64 * k : 64 * (k + 1)
                    ],
                    ops[:, :].rearrange("q (p x) -> q x p", x=O3),
                    b3bc[:, :].rearrange("q (x p) -> q x p", x=O3)[
                        :, :, 64 * k : 64 * (k + 1)
                    ],
                    ADD,
                )
                if k == 1 or k == 3:
                    nc.scalar.dma_start(
                        out=OUT[:, :, 128 * (k // 2) : 128 * (k // 2 + 1)],
                        in_=out_sb[:, :].rearrange("q (x p) -> q x p", x=O3)[
                            :, :, 128 * (k // 2) : 128 * (k // 2 + 1)
                        ],
                    )

            # ---------------- software pipeline ----------------
            for t in range(NT):
                dma_w1(t)
            psts = {}
            for t in range(NT + 2):
                if t < NT:
                    psts[t] = stage1(t)
                    drain(t, psts[t])
                if t >= 1 and t - 1 < NT:
                    stage2(t - 1)
                if t >= 2:
                    tt = t - 2
                    stage3(tt)
                    if tt % 4 == 3:
                        finish(tt // 4)

    nc.compile()
    return nc


def _get_nc():
    if "nc" not in _CACHE:
        _CACHE["nc"] = _build_nc()
    return _CACHE["nc"]


def _make_in_maps(inputs):
    glf = np.asarray(inputs["glf"], dtype=np.float32).reshape(B, I)
    # glfT packed [q=128, c=4, b=32]: glfT[q, c, b] = glf[b, 128c + q]
    glft = np.ascontiguousarray(
        glf.T.reshape(4, 128, B).transpose(1, 0, 2).reshape(128, 4 * B)
    ).astype(np.float16)
    ins = {k: np.asarray(inputs[k], dtype=np.float32) for k in
           ("W1", "b1", "W2", "b2", "W3", "b3")}
    in_maps = []
    for c in range(NCORES):
        sl = slice(c * PP, (c + 1) * PP)
        W1c = ins["W1"][sl]  # [256, 32, 512]
        # [NT=16, q=128, (u=4, i=512)];  rows of W1 flat [(p,o1), i] grouped as
        # t-units of 512 rows, u-major within q.
        w1h = np.ascontiguousarray(
            W1c.reshape(NT, 4, 128, I).transpose(0, 2, 1, 3).reshape(NT, 128, 4 * I)
        ).astype(np.float16)
        w2t = np.ascontiguousarray(
            ins["W2"][sl].transpose(2, 0, 1).reshape(O1, PP * O2)
        ).astype(np.float16)
        # block-diag W3T: w3bd[8*pl + o2, 48*t + 3*pl + o3] = W3[16t + pl, o3, o2]
        W3c = ins["W3"][sl].astype(np.float16)  # [256, 3, 8]
        w3bd = np.zeros((16, O2, NT, 16, O3), dtype=np.float16)
        for pl in range(16):
            # [o2, t, o3] slice for this within-group position
            w3bd[pl, :, :, pl, :] = W3c.reshape(NT, 16, O3, O2)[
                :, pl, :, :
            ].transpose(2, 0, 1)
        w3bd = np.ascontiguousarray(w3bd.reshape(128, NT * 48))
        b1c = np.ascontiguousarray(
            ins["b1"][sl].reshape(64, 4, O1).transpose(1, 2, 0).reshape(128, 64)
        ).astype(np.float16)
        b2r = ins["b2"][sl].reshape(1, PP * O2).astype(np.float16)
        b3r = np.ascontiguousarray(ins["b3"][sl].T.reshape(1, O3 * PP))
        in_maps.append(
            {
                "W1h": w1h,
                "W2T": w2t,
                "W3BD": w3bd,
                "GLFT": glft,
                "B1C": b1c,
                "B2R": b2r,
                "B3R": b3r,
            }
        )
    return in_maps


def run(inputs, trace=False):
    """Run on the 8 NeuronCores; returns (out_full, BassKernelResults)."""
    from concourse.bass_utils import run_bass_kernel_spmd

    nc = _get_nc()
    res = run_bass_kernel_spmd(
        nc, _make_in_maps(inputs), list(range(NCORES)), trace=trace
    )
    out_full = np.empty((B, O3, P_FULL), dtype=np.float32)
    for c in range(NCORES):
        out_full[:, :, c * PP : (c + 1) * PP] = res.results[c]["OUT"]
    return out_full, res


def kernel(**inputs):
    out, _ = run(inputs, trace=False)
    return out
